# revision 9
# baseline (speedup 1.0000x reference)
"""DRQConv2d (dual-region quantized conv) Trainium2 kernel, v3.

Reference semantics:
  mask  = upsample8(avgpool8(x) >= 0.05)             per (b, c)
  xh    = where(mask, x, 1e-5);  xl = where(mask, 1e-5, x)
  qh    = clip(round(xh/sh), 0, 255) * sh            (uint8 fake-quant)
  ql    = clip(round(xl/sl), 0, 15) * sl             (uint4 fake-quant)
  qwh   = per-oc quant of w_high to +-127,  qwl = per-oc quant of w_low to +-7
  y     = conv3x3(qh, qwh) + conv3x3(ql, qwl)        (pad 1)

v3 design (baseline 154us -> v2 126us -> target ~100us):
  * Host-side weight quantization (exact); device only DMAs prepped weights.
  * High conv: bf16 integer weights, 9 restricted-region matmuls per chunk
    (no padded layout, per-element PSUM has_written handles borders).
  * Low conv: fp8 e4m3 (ints exact); taps (0,kw)+(2,kw) pair into DoubleRow
    matmuls via an in-tile j-stride of 112B; taps (1,kw) are normal fp8.
  * Engine assignment to keep every queue under the PE roofline:
      ACT    relu/scale quant front-end; PSUM*scale evacuation reads
      DVE    min+round-magic (in-place ts), masked un-magic STT (2D, one per
             path per image)
      GPSIMD entire mask pipeline (pairwise-add pooling, threshold, full-res
             bf16 mask expansion) + final evac add (SBUF only)
      PE     convs + warm-up stream
  * Software pipelining: image b+1's prep is EMITTED before image b's conv
    chunks so the in-order ACT/DVE/GPSIMD queues never trap next-image prep
    behind conv-dependent work.
  * Image 0 is processed in 16-row DMA bands; x0 bands are posted before the
    weight DMAs so the first conv matmul fires at ~13us.

Sharding: data-parallel over batch.  32 images -> 4 per core on 8 cores,
weights replicated; outputs concatenated on host.  No collectives.
"""

import numpy as np
import ml_dtypes

P = 128            # channels (both in and out) == partitions
B_TOTAL = 32
N_CORES = 8
BPC = B_TOTAL // N_CORES   # images per core
H = W = 56
NPIX = H * W       # 3136
NTAPS = 9
RPC = 8                       # output rows per chunk
NCHUNK = H // RPC             # 7
NFREE = RPC * W               # 448 psum columns per chunk
MAGIC = float(np.float32(1.5 * 2 ** 23))   # fp32 round-to-nearest magic
POOL_K = 8
THRESH = 0.05
BAND_ROWS = [8, 8, 16, 24]    # image-0 DMA bands (rows)
WARM_MMS = 14


# ---------------------------------------------------------------- host side

def _host_weight_prep(w, n):
    """Quantize per-oc exactly like the reference (fp32 divide + round-half-
    even + clip).  Returns integer weights [oc, ic, 9] (as fp32) and the
    per-oc weight scale s = absmax/n (fp32)."""
    w = np.asarray(w, dtype=np.float32).reshape(P, P, NTAPS)
    absmax = np.abs(w.reshape(P, -1)).max(axis=1).astype(np.float32)
    s = (absmax / np.float32(n)).astype(np.float32)
    ratio = w / s[:, None, None]          # fp32, like the reference
    wint = np.clip(np.round(ratio), -n, n).astype(np.float32)
    return wint, s


def _prep_inputs(w_high, w_low, act_scale_high, act_scale_low):
    sh = float(np.float32(act_scale_high))
    sl = float(np.float32(act_scale_low))
    inv_sh = float(np.float32(1.0 / np.float64(sh)))
    inv_sl = float(np.float32(1.0 / np.float64(sl)))

    wih, s_h = _host_weight_prep(w_high, 127.0)
    wil, s_l = _host_weight_prep(w_low, 7.0)

    bf16 = ml_dtypes.bfloat16
    e4 = ml_dtypes.float8_e4m3

    # high: [ic, tap, oc] bf16 integers (exact)
    qwt_h = np.ascontiguousarray(
        wih.transpose(1, 2, 0).astype(bf16)).reshape(P, NTAPS * P)
    # low pairs: [ic, kw, j, oc] fp8, j=0 -> tap (0,kw), j=1 -> tap (2,kw)
    wil_t = wil.transpose(1, 2, 0)        # [ic, tap, oc]
    pairs = np.stack([
        np.stack([wil_t[:, kw, :], wil_t[:, 6 + kw, :]], axis=1)
        for kw in range(3)], axis=1)      # [ic, kw, j, oc]
    qwt_l_p = np.ascontiguousarray(pairs.astype(e4)).reshape(P, 3 * 2 * P)
    # low singles: [ic, kw, oc] fp8  (taps (1,kw))
    qwt_l_s = np.ascontiguousarray(
        wil_t[:, 3:6, :].astype(e4)).reshape(P, 3 * P)

    # full output scales per oc (fp32)
    sv_h = (np.float64(sh) * s_h.astype(np.float64)).astype(np.float32)
    sv_l = (np.float64(sl) * s_l.astype(np.float64)).astype(np.float32)

    return {
        "qwt_h": qwt_h,
        "qwt_l_p": qwt_l_p,
        "qwt_l_s": qwt_l_s,
        "sv_h": sv_h.reshape(P, 1),
        "sv_l": sv_l.reshape(P, 1),
    }, inv_sh, inv_sl


# ---------------------------------------------------------------- device side

def build_program(nc, tc, aps, inv_sh, inv_sl, bpc=BPC):
    import concourse.mybir as mybir
    from concourse.alu_op_type import AluOpType as op

    f32 = mybir.dt.float32
    bf16 = mybir.dt.bfloat16
    fp8 = mybir.dt.float8e4
    DR = mybir.MatmulPerfMode.DoubleRow

    x_d, y_d = aps["x"], aps["y"]
    sum_thresh = float(np.float32(THRESH) * POOL_K * POOL_K)  # exact pow2 scale

    with (
        tc.tile_pool(name="consts", bufs=1) as consts,
        tc.tile_pool(name="xs", bufs=3) as xs_pool,
        tc.tile_pool(name="rs", bufs=3) as rs_pool,
        tc.tile_pool(name="qs", bufs=3) as qs_pool,
        tc.tile_pool(name="mk", bufs=2) as mk_pool,
        tc.tile_pool(name="ev", bufs=4) as ev_pool,
        tc.tile_pool(name="cps", bufs=4, space="PSUM") as cps,
    ):
        # ---- weights / scales (host-prepped, DMA only)
        qwt_h = consts.tile([P, NTAPS * P], bf16, tag="qwt_h")
        qwt_l_p = consts.tile([P, 6 * P], fp8, tag="qwt_l_p")
        qwt_l_s = consts.tile([P, 3 * P], fp8, tag="qwt_l_s")
        sv_h = consts.tile([P, 1], f32, tag="sv_h")
        sv_l = consts.tile([P, 1], f32, tag="sv_l")

        # ---- PE warm-up: no data deps; runs while DMAs stream in.
        warm_l = consts.tile([P, P], bf16, tag="warm_l")
        warm_r = consts.tile([P, NFREE], bf16, tag="warm_r")
        nc.gpsimd.memset(warm_l[:], 0.0)
        nc.gpsimd.memset(warm_r[:], 0.0)
        warm_ps = cps.tile([P, NFREE], f32, tag="ps_h", name="warm")
        for i in range(WARM_MMS):
            nc.tensor.matmul(
                warm_ps[:], warm_l[:], warm_r[:],
                start=(i == 0), stop=(i == WARM_MMS - 1),
            )

        # ---- input DMAs; order sets HBM arrival priority for the head
        xts = {}
        xts[0] = xs_pool.tile([P, NPIX], f32, tag="xt", name="xt0")
        for b in range(1, bpc):
            xts[b] = xs_pool.tile([P, NPIX], f32, tag="xt", name=f"xt{b}")

        def band_dma(b, row0, rows):
            nc.sync.dma_start(
                out=xts[b][:, row0 * W:(row0 + rows) * W],
                in_=x_d[b][:, row0 * W:(row0 + rows) * W],
            )

        nc.sync.dma_start(out=qwt_h[:], in_=aps["qwt_h"])
        band_dma(0, 0, 8)
        band_dma(0, 8, 8)
        nc.sync.dma_start(out=qwt_l_p[:], in_=aps["qwt_l_p"])
        nc.sync.dma_start(out=qwt_l_s[:], in_=aps["qwt_l_s"])
        nc.sync.dma_start(out=sv_h[:], in_=aps["sv_h"])
        nc.sync.dma_start(out=sv_l[:], in_=aps["sv_l"])
        band_dma(0, 16, 16)
        if bpc > 1:
            nc.sync.dma_start(out=xts[1][:], in_=x_d[1])
        band_dma(0, 32, 24)
        for b in range(2, bpc):
            nc.sync.dma_start(out=xts[b][:], in_=x_d[b])

        def image_tiles(b):
            t1 = mk_pool.tile([P, 1568], f32, tag="t1", name=f"t1_{b}")
            t2 = mk_pool.tile([P, 784], f32, tag="t2", name=f"t2_{b}")
            sc = mk_pool.tile([P, 392], f32, tag="sc", name=f"sc_{b}")
            u1 = mk_pool.tile([P, 196], f32, tag="u1", name=f"u1_{b}")
            u2 = mk_pool.tile([P, 98], f32, tag="u2", name=f"u2_{b}")
            s2 = mk_pool.tile([P, 49], f32, tag="s2", name=f"s2_{b}")
            ml = mk_pool.tile([P, 49], f32, tag="ml", name=f"ml_{b}")
            mw_h = mk_pool.tile([P, 392], bf16, tag="mw_h", name=f"mwh_{b}")
            mw_l = mk_pool.tile([P, 392], bf16, tag="mw_l", name=f"mwl_{b}")
            r_h = rs_pool.tile([P, NPIX], f32, tag="r", name=f"rh_{b}")
            r_l = rs_pool.tile([P, NPIX], f32, tag="r", name=f"rl_{b}")
            t_h = rs_pool.tile([P, NPIX], bf16, tag="t16", name=f"th_{b}")
            t_l = rs_pool.tile([P, NPIX], bf16, tag="t16", name=f"tl_{b}")
            qh = qs_pool.tile([P, NPIX], bf16, tag="qh", name=f"qh_{b}")
            ql = qs_pool.tile([P, NPIX], fp8, tag="ql", name=f"ql_{b}")
            return (t1, t2, sc, u1, u2, s2, ml, mw_h, mw_l), \
                (r_h, r_l, t_h, t_l), qh, ql

        def mask_ops(xt, mt, hb0, nhb, g):
            """Mask pipeline for hb rows [hb0, hb0+nhb): pairwise-add pooling,
            threshold, row-pattern expansion to [P, hb*56].  Runs on `g`
            (DVE for image 0 to cut head latency, GPSIMD for the rest)."""
            t1, t2, sc, u1, u2, s2, ml, mw_h, mw_l = mt
            rows = nhb * POOL_K

            def halve(dst, src, n_out):
                v = src.rearrange("p (g c) -> p g c", c=2)
                g.tensor_tensor(out=dst, in0=v[:, :, 0], in1=v[:, :, 1],
                                op=op.add)

            # columns: 8 -> 1 per block (3 rounds)
            halve(t1[:, hb0 * 224:(hb0 + nhb) * 224],
                  xt[:, hb0 * 448:(hb0 + nhb) * 448], rows * 28)
            halve(t2[:, hb0 * 112:(hb0 + nhb) * 112],
                  t1[:, hb0 * 224:(hb0 + nhb) * 224], rows * 14)
            halve(sc[:, hb0 * 56:(hb0 + nhb) * 56],
                  t2[:, hb0 * 112:(hb0 + nhb) * 112], rows * 7)
            # rows: 8 -> 1 per block (3 rounds); sc is [row, wb] row-major
            v = sc[:, hb0 * 56:(hb0 + nhb) * 56].rearrange(
                "p (r w) -> p r w", w=7)
            g.tensor_tensor(out=u1[:, hb0 * 28:(hb0 + nhb) * 28],
                            in0=v[:, 0:rows:2, :], in1=v[:, 1:rows:2, :],
                            op=op.add)
            v = u1[:, hb0 * 28:(hb0 + nhb) * 28].rearrange(
                "p (r w) -> p r w", w=7)
            g.tensor_tensor(out=u2[:, hb0 * 14:(hb0 + nhb) * 14],
                            in0=v[:, 0:rows // 2:2, :],
                            in1=v[:, 1:rows // 2:2, :], op=op.add)
            v = u2[:, hb0 * 14:(hb0 + nhb) * 14].rearrange(
                "p (r w) -> p r w", w=7)
            g.tensor_tensor(out=s2[:, hb0 * 7:(hb0 + nhb) * 7],
                            in0=v[:, 0:rows // 4:2, :],
                            in1=v[:, 1:rows // 4:2, :], op=op.add)
            # threshold -> {0,1}; low mask = 1 - m
            g.tensor_scalar(
                s2[:, hb0 * 7:(hb0 + nhb) * 7], s2[:, hb0 * 7:(hb0 + nhb) * 7],
                sum_thresh, None, op0=op.is_ge)
            g.tensor_scalar(
                ml[:, hb0 * 7:(hb0 + nhb) * 7], s2[:, hb0 * 7:(hb0 + nhb) * 7],
                -1.0, 1.0, op0=op.mult, op1=op.add)
            # expand wb -> 8 cols: row-pattern masks [P, hb*56] (f32)
            for src, dst in ((s2, mw_h), (ml, mw_l)):
                g.tensor_copy(
                    out=dst[:, hb0 * 56:(hb0 + nhb) * 56].rearrange(
                        "p (g c) -> p g c", c=POOL_K),
                    in_=src[:, hb0 * 7:(hb0 + nhb) * 7].unsqueeze(2)
                    .broadcast_to((P, nhb * 7, POOL_K)),
                )

        def quant_front(xt, r, t16, inv_s, row0, rows):
            """relu/scale (ACT) -> round-to-int via magic add/sub, bf16 out
            (DVE)."""
            sl_ = slice(row0 * W, (row0 + rows) * W)
            nc.scalar.activation(
                r[:, sl_], xt[:, sl_],
                mybir.ActivationFunctionType.Relu, scale=inv_s,
            )
            nc.vector.tensor_scalar(
                t16[:, sl_], r[:, sl_], MAGIC, MAGIC,
                op0=op.add, op1=op.subtract,
            )

        def quant_stt(t16, q, mw, qmax, hb):
            """clip + masked multiply, all-16-bit STT (2x DVE rate)."""
            in1 = mw[:, hb * 56:(hb + 1) * 56].unsqueeze(1)
            in1 = in1.broadcast_to((P, POOL_K, W))
            t3 = t16[:].rearrange("p (r c) -> p r c", c=W)
            q3 = q[:].rearrange("p (r c) -> p r c", c=W)
            nc.vector.scalar_tensor_tensor(
                out=q3[:, hb * POOL_K:(hb + 1) * POOL_K, :],
                in0=t3[:, hb * POOL_K:(hb + 1) * POOL_K, :],
                scalar=qmax, in1=in1, op0=op.min, op1=op.mult,
            )

        def conv_chunk(b, c, qh, ql):
            """All 18 taps for output rows [8c, 8c+8) + evacuation + store."""
            r0 = c * RPC
            ps_h = cps.tile([P, NFREE], f32, tag="ps_h", name=f"psh{b}_{c}")
            ps_l = cps.tile([P, NFREE], f32, tag="ps_l", name=f"psl{b}_{c}")
            ph3 = ps_h[:].rearrange("p (r c) -> p r c", c=W)
            pl3 = ps_l[:].rearrange("p (r c) -> p r c", c=W)
            qh3 = qh[:].rearrange("p (r c) -> p r c", c=W)
            ql3 = ql[:].rearrange("p (r c) -> p r c", c=W)

            def region(kh, kw):
                rlo = max(r0, 1 - kh)             # kh=0 -> >=1
                rhi = min(r0 + RPC - 1, 56 - kh)  # kh=2 -> <=54
                clo = max(0, 1 - kw)
                chi = min(W - 1, 56 - kw)
                return rlo, rhi, clo, chi

            # ---- high conv (bf16, 9 taps; center tap first, full coverage)
            taps = [(1, 1)] + [(kh, kw) for kh in range(3) for kw in range(3)
                               if (kh, kw) != (1, 1)]
            for i, (kh, kw) in enumerate(taps):
                rlo, rhi, clo, chi = region(kh, kw)
                nr, ncl = rhi - rlo + 1, chi - clo + 1
                nc.tensor.matmul(
                    ph3[:, rlo - r0:rlo - r0 + nr, clo:clo + ncl],
                    qwt_h[:, (kh * 3 + kw) * P:(kh * 3 + kw + 1) * P],
                    qh3[:, rlo + kh - 1:rlo + kh - 1 + nr,
                        clo + kw - 1:clo + kw - 1 + ncl],
                    start=(i == 0), stop=(i == len(taps) - 1),
                )

            # ---- low conv (fp8): singles (1,kw) then DR pairs then minis
            n_low = 6 + (3 if c in (0, NCHUNK - 1) else 0)
            li = 0
            for kw in (1, 0, 2):
                rlo, rhi, clo, chi = region(1, kw)
                nr, ncl = rhi - rlo + 1, chi - clo + 1
                nc.tensor.matmul(
                    pl3[:, rlo - r0:rlo - r0 + nr, clo:clo + ncl],
                    qwt_l_s[:, kw * P:(kw + 1) * P],
                    ql3[:, rlo:rlo + nr, clo + kw - 1:clo + kw - 1 + ncl],
                    start=(li == 0), stop=(li == n_low - 1),
                )
                li += 1
            for kw in range(3):
                rlo = max(r0, 1)
                rhi = min(r0 + RPC - 1, 54)
                clo = max(0, 1 - kw)
                chi = min(W - 1, 56 - kw)
                nr, ncl = rhi - rlo + 1, chi - clo + 1
                rhs = ql3[:, rlo - 1:rlo - 1 + nr,
                          clo + kw - 1:clo + kw - 1 + ncl]
                rhs = rhs.unsqueeze(1).broadcast_to((P, 2, nr, ncl))
                rhs.ap = mybir.VecI64Pair(
                    [[NPIX, P], [2 * W, 2], [W, nr], [1, ncl]])
                nc.tensor.matmul(
                    pl3[:, rlo - r0:rlo - r0 + nr, clo:clo + ncl],
                    qwt_l_p[:, kw * 2 * P:(kw + 1) * 2 * P].rearrange(
                        "p (j m) -> p j m", j=2),
                    rhs,
                    start=False, stop=(li == n_low - 1),
                    perf_mode=DR,
                )
                li += 1
            if c == 0:
                # out row 0 misses taps (2,kw): input row 1
                for kw in range(3):
                    clo, chi = max(0, 1 - kw), min(W - 1, 56 - kw)
                    ncl = chi - clo + 1
                    nc.tensor.matmul(
                        pl3[:, 0:1, clo:clo + ncl],
                        qwt_l_p[:, (kw * 2 + 1) * P:(kw * 2 + 2) * P],
                        ql3[:, 1:2, clo + kw - 1:clo + kw - 1 + ncl],
                        start=False, stop=(li == n_low - 1),
                    )
                    li += 1
            elif c == NCHUNK - 1:
                # out row 55 misses taps (0,kw): input row 54
                for kw in range(3):
                    clo, chi = max(0, 1 - kw), min(W - 1, 56 - kw)
                    ncl = chi - clo + 1
                    nc.tensor.matmul(
                        pl3[:, RPC - 1:RPC, clo:clo + ncl],
                        qwt_l_p[:, (kw * 2) * P:(kw * 2 + 1) * P],
                        ql3[:, 54:55, clo + kw - 1:clo + kw - 1 + ncl],
                        start=False, stop=(li == n_low - 1),
                    )
                    li += 1

            # ---- evacuate:  y = ps_h*sv_h + ps_l*sv_l
            # (ACT scales the low bank; DVE STT merges with the high bank)
            tmp = ev_pool.tile([P, NFREE], f32, tag="tmp")
            nc.scalar.mul(tmp[:], ps_l[:], sv_l[:, 0:1])
            acc = ev_pool.tile([P, NFREE], f32, tag="acc")
            nc.vector.scalar_tensor_tensor(
                out=acc[:], in0=ps_h[:], scalar=sv_h[:, 0:1], in1=tmp[:],
                op0=op.mult, op1=op.add,
            )
            nc.sync.dma_start(
                out=y_d[b][:, r0 * W:(r0 + RPC) * W], in_=acc[:],
            )

        # ---------------- banded prep (all images) ----------------
        prep = {}

        def prep_bands(b, bands, mask_eng):
            mt, (r_h, r_l, t_h, t_l), qh, ql = prep[b]
            hb0 = 0
            for rows in bands:
                nhb = rows // POOL_K
                row0 = hb0 * POOL_K
                mask_ops(xts[b], mt, hb0, nhb, mask_eng)
                quant_front(xts[b], r_h, t_h, inv_sh, row0, rows)
                quant_front(xts[b], r_l, t_l, inv_sl, row0, rows)
                for hb in range(hb0, hb0 + nhb):
                    quant_stt(t_h, qh, mt[7], 255.0, hb)
                    quant_stt(t_l, ql, mt[8], 15.0, hb)
                hb0 += nhb

        prep[0] = image_tiles(0)
        prep_bands(0, BAND_ROWS, nc.vector)

        # image 1 prep BEFORE image 0 chunks (pipelining: keeps next-image
        # prep ahead of conv-dependent work in the in-order engine queues)
        if bpc > 1:
            prep[1] = image_tiles(1)
            prep_bands(1, [24, 32], nc.gpsimd)

        for b in range(bpc):
            _, _, qh, ql = prep[b]
            for c in range(NCHUNK):
                conv_chunk(b, c, qh, ql)
            nb = b + 2
            if nb < bpc:
                prep[nb] = image_tiles(nb)
                prep_bands(nb, [24, 32], nc.gpsimd)


def make_bass(inv_sh, inv_sl, bpc=BPC):
    import concourse.bacc as bacc
    import concourse.mybir as mybir
    from concourse.tile import TileContext

    f32 = mybir.dt.float32
    bf16 = mybir.dt.bfloat16
    fp8 = mybir.dt.float8e4
    nc = bacc.Bacc("TRN2", debug=False)
    x = nc.dram_tensor("x", [bpc, P, NPIX], f32, kind="ExternalInput")
    qwh = nc.dram_tensor("qwt_h", [P, NTAPS * P], bf16, kind="ExternalInput")
    qwlp = nc.dram_tensor("qwt_l_p", [P, 6 * P], fp8, kind="ExternalInput")
    qwls = nc.dram_tensor("qwt_l_s", [P, 3 * P], fp8, kind="ExternalInput")
    svh = nc.dram_tensor("sv_h", [P, 1], f32, kind="ExternalInput")
    svl = nc.dram_tensor("sv_l", [P, 1], f32, kind="ExternalInput")
    y = nc.dram_tensor("y", [bpc, P, NPIX], f32, kind="ExternalOutput")
    aps = {
        "x": x.ap(), "y": y.ap(),
        "qwt_h": qwh.ap(), "qwt_l_p": qwlp.ap(), "qwt_l_s": qwls.ap(),
        "sv_h": svh.ap(), "sv_l": svl.ap(),
    }
    with TileContext(nc) as tc:
        build_program(nc, tc, aps, inv_sh, inv_sl, bpc=bpc)
    nc.compile()
    return nc


def _run(x, w_high, w_low, act_scale_high, act_scale_low, trace=False, **kw):
    from concourse import bass_utils

    x = np.ascontiguousarray(np.asarray(x, dtype=np.float32))
    w_high = np.asarray(w_high, dtype=np.float32)
    w_low = np.asarray(w_low, dtype=np.float32)

    wmap, inv_sh, inv_sl = _prep_inputs(
        w_high, w_low, act_scale_high, act_scale_low)
    nc = make_bass(inv_sh, inv_sl)

    in_maps = []
    for core in range(N_CORES):
        xs = x[core * BPC:(core + 1) * BPC].reshape(BPC, P, NPIX)
        m = {"x": np.ascontiguousarray(xs)}
        m.update(wmap)
        in_maps.append(m)
    res = bass_utils.run_bass_kernel_spmd(
        nc, in_maps, core_ids=list(range(N_CORES)), trace=trace, **kw
    )
    y = np.concatenate([r["y"].reshape(BPC, P, H, W) for r in res.results], axis=0)
    return y, res


def kernel(x, w_high, w_low, act_scale_high, act_scale_low):
    y, _ = _run(x, w_high, w_low, act_scale_high, act_scale_low)
    return y
